# revision 41
# baseline (speedup 1.0000x reference)
"""Gaussian resampling kernel for Trainium2 (8 NeuronCores, SPMD).

Computes, for each batch row b:
    e = cumsum(d); c = e - d/2
    w[t, s] = softmax_s(-(t - c_s)^2 / 10)   (masked s get weight 0)
    out[t, :] = sum_s w[t, s] * x[s, :]

Strategy (fp16 num/den output, host-side normalization):
  - Host precomputes c (float64 cumsum) and folds the mask in by moving
    masked centers to -1e4 (their exp underflows to exactly 0 in fp32).
  - Data-parallel over batch: 2 batches per core on 8 cores, sorted by
    valid length and paired into per-core slots of similar length.
  - Scores in [S, T] layout (tokens on partitions): ONE ACT pass per
    piece using Derivative_Erf (d/dx erf = 2/sqrt(pi) * exp(-x^2); the
    constant factor cancels in the host-side num/den divide), emitting
    fp16 directly. Banded sparsity: each 128-token chunk is only active
    in a contiguous frame range (union over the slot's 8 batches, baked
    into the program).
  - x is scaled by 256 and a 256-column appended: the matmul produces
    256*numerator (T, D) and 256*denominator (T, 1) together, both in
    fp16 normal range (no subnormal precision loss). The softmax divide
    happens on the HOST after the gather — the device never normalizes,
    so PSUM evacuation is a pure dtype-converting copy.
  - PSUM is ONE ring tile [128, RING=4, 1024] f32 (2 banks per slot);
    the chunk at sequence position ci accumulates in slot ci%RING, so
    up to 4 output chunks are in flight (PE runs ahead of evacuation;
    slice-level WAR deps recycle slots). Each chunk is evacuated
    PSUM->SBUF fp16 by ONE copy op, assigned to DVE (tensor_copy) or
    ACT (activation Copy) by a greedy balancer that also charges ACT
    for its Derivative_Erf score pieces — both engines finish together.
  - Output DMA per 4-chunk group (512 frames, one Sync op); the final
    group is split in half so its wire starts earlier. Score pieces are
    issued just-in-time (PIECE_LEAD chunk positions before first use).
  - Junk matmuls at startup warm the PE clock gate (HAM pstate) — they
    target the LAST ring slot so real chunks don't queue behind them;
    without them the PE never reaches 2.4GHz and the whole pipeline
    slows ~15%. Frame indices: first 512 DMA'd from the host, then
    cheap DVE doubling adds extend to T.
"""

import math
import sys
import types

import numpy as np

# ---------------------------------------------------------------------------
# Optional NTFF-profiling plumbing. The runtime image lacks
# antenv.axon_hooks; wire a stand-in so run_bass_kernel_spmd(trace=True)
# works (used by the dev harness; the plain kernel path never traces).
try:  # pragma: no cover - best effort
    import antenv.axon_hooks  # noqa: F401
except ImportError:
    try:
        _hooks_mod = types.ModuleType("antenv.axon_hooks")
        _hook_box = [None]
        _hooks_mod.set_axon_ntff_profile_hook = (
            lambda hook: _hook_box.__setitem__(0, hook)
        )
        _hooks_mod.get_axon_ntff_profile_hook = lambda: _hook_box[0]
        sys.modules["antenv.axon_hooks"] = _hooks_mod
        from trn_agent_boot.trn_boot import _ntff_profile_via_ctypes

        _hooks_mod.set_axon_ntff_profile_hook(
            _ntff_profile_via_ctypes("/opt/axon/libaxon_pjrt.so")
        )
    except Exception:
        pass

import concourse.bacc as bacc
import concourse.mybir as mybir
import concourse.tile as tile
import concourse.bass_utils as bass_utils
from concourse.tile_rust import add_dep_helper

# Avoid S3 artifact uploads from the trace path in this container.
bass_utils.upload_artifacts = lambda tmpdir: f"local:{tmpdir}"

from concourse.bass_utils import run_bass_kernel_spmd

NCORES = 8
B, S, D, T = 16, 512, 768, 4096
VARIANCE = 10.0
BPC = B // NCORES          # batches per core
P = 128                    # partitions
KC = S // P                # token chunks (4)
MC = T // P                # output frame chunks (32)
QC = MC // 2               # output pair-chunks per slot (16)
DW = D + 1                 # x with the scaled-ones column appended
N0 = 512                   # first matmul column split (one PSUM bank)
XSCALE = 256.0             # keeps fp16 num/den in normal range
MARGIN = 14.0              # frames; exp(-14^2/10) underflows fp16 to 0
ACT_PIECE = 2048           # max free-dim length of one score ACT op
PIECE_LEAD = 8             # issue score pieces this many chunks early
GRP = 4                    # 128-frame chunks per output DMA group
RING = 4                   # PSUM ring slots (2 banks each)

# chunk-evac cost model (ns) for the greedy DVE/ACT balancer
COST_V = 1160.0            # DVE tensor_copy [128, 769] PSUM->SBUF + sem
COST_A = 1140.0            # ACT activation-Copy [128, 769] PSUM->SBUF + sem
COST_PIECE_BASE = 190.0    # ACT activation fixed cost
COST_PIECE_EL = 0.84       # ACT per-element (fp32 in)

_PROGRAMS = {}


def _compute_bands(c_masked):
    """Per token-chunk active frame range, unioned over the given
    batches. Returns (a, b, lo, hi): [a, b) is 128-aligned (score-piece
    extent), [lo, hi) is the tight range (beyond it every score in the
    chunk underflows to exactly 0 in fp16, so matmuls can be skipped).
    c_masked: (n, S) float64, masked tokens nan. A fully-masked chunk
    yields None (skipped entirely)."""
    bands = []
    for k in range(KC):
        ck = c_masked[:, k * P:(k + 1) * P]
        if np.all(np.isnan(ck)):
            bands.append(None)
            continue
        lo = np.nanmin(ck) - MARGIN
        hi = np.nanmax(ck) + MARGIN
        a = max(0, int(math.floor(lo - 1)) // P * P)
        b = min(T, -(-int(math.ceil(hi)) // P) * P)
        b = max(b, a + P)
        bands.append((a, b, max(a, int(math.floor(lo))),
                      min(b, int(math.ceil(hi)))))
    return tuple(bands)


def _build_program(bands2):
    """bands2: per batch-slot tuple of per-chunk (a, b) bands (or None)."""
    nc = bacc.Bacc("TRN2", target_bir_lowering=False, debug=False)
    f32 = mybir.dt.float32
    fp16 = mybir.dt.float16

    xw_d = nc.dram_tensor("xw", [BPC, S, DW], fp16, kind="ExternalInput").ap()
    bias_d = nc.dram_tensor("bias", [BPC, P, KC], f32,
                        kind="ExternalInput").ap()
    trow0_d = nc.dram_tensor("trow0", [P, 512], f32,
                             kind="ExternalInput").ap()
    out_d = nc.dram_tensor("out", [BPC, T, DW], fp16,
                           kind="ExternalOutput").ap()

    rsv = 1.0 / math.sqrt(VARIANCE)
    AF = mybir.ActivationFunctionType

    # score pieces (k, t0, t1) in frame order; matmul chunk lists per m.
    # Pieces cover the 128-aligned band [a, b); matmul inclusion uses the
    # TIGHT band [lo, hi) — outside it the chunk's scores are exactly 0
    # in fp16, so boundary matmuls that exist only due to 128-rounding
    # are dropped (bitwise-identical result, fewer PE ops).
    pieces2, mk2 = [], []
    for bands in bands2:
        pieces = []
        for k, band in enumerate(bands):
            if band is None:
                continue
            a, b = band[0], band[1]
            t0 = a
            while t0 < b:
                t1 = min(t0 + ACT_PIECE, b)
                pieces.append((k, t0, t1))
                t0 = t1
        pieces.sort(key=lambda p: (p[1], p[0]))
        if pieces and pieces[0][2] - pieces[0][1] > 512:
            k, t0, t1 = pieces[0]
            pieces[0:1] = [(k, t0, t0 + 256), (k, t0 + 256, t0 + 512),
                           (k, t0 + 512, t1)]
        pieces2.append(pieces)
        mk = []
        for m in range(MC):
            ks = [k for k, band in enumerate(bands)
                  if band and m * P < band[3] and (m + 1) * P > band[2]]
            assert ks, f"no active token chunk for m={m}"
            mk.append(ks)
        mk2.append(mk)

    # Group sequence (GRP 128-frame chunks per group): slot 0 leads while
    # slot 1's scores are still being produced, then the slots interleave.
    NG = MC // GRP
    H = NG // 2
    group_seq = [(0, g) for g in range(H)]
    for i in range(H):
        group_seq.append((0, H + i))
        group_seq.append((1, i))
    group_seq += [(1, g) for g in range(H, NG)]
    # chunk sequence: (b, m) in execution order
    chunk_seq = []
    for b, g in group_seq:
        for s in range(GRP):
            chunk_seq.append((b, g * GRP + s))

    # Just-in-time piece schedule: issue each piece PIECE_LEAD chunk
    # positions before the first chunk that consumes its scores.
    first_use = {}
    for ci, (b, m) in enumerate(chunk_seq):
        for k in mk2[b][m]:
            lo, hi = m * P, (m + 1) * P
            for pi, (pk, t0, t1) in enumerate(pieces2[b]):
                if pk == k and t0 < hi and t1 > lo:
                    first_use.setdefault((b, pi), ci)
    issue_at = {}
    for (b, pi), use in sorted(first_use.items()):
        issue_at.setdefault(max(0, use - PIECE_LEAD), []).append((b, pi))

    # Greedy engine assignment for per-chunk evacuation: pick the engine
    # with the smaller projected finish, charging ACT for score pieces.
    eng_seq = {}
    act_load = 0.0
    dve_load = 0.0
    for ci in range(len(chunk_seq)):
        for pb, pi in issue_at.get(ci, ()):
            k, t0, t1 = pieces2[pb][pi]
            act_load += COST_PIECE_BASE + COST_PIECE_EL * (t1 - t0)
        if act_load + COST_A <= dve_load + COST_V:
            eng_seq[ci] = "A"
            act_load += COST_A
        else:
            eng_seq[ci] = "V"
            dve_load += COST_V

    with tile.TileContext(nc) as tc:
        with tc.tile_pool(name="const", bufs=1) as constp, \
             tc.tile_pool(name="sb", bufs=2) as sb, \
             tc.tile_pool(name="outp", bufs=10) as outp, \
             tc.tile_pool(name="colp", bufs=4) as colp, \
             tc.tile_pool(name="ps", bufs=4, space="PSUM") as ps:

            # Warm the ACT table set (erf_derivative; also holds Copy)
            # before any real work.
            warm = colp.tile([P, 1], f32, name="warm", tag="warm", bufs=1)
            nc.vector.memset(warm[:], 0.0)
            nc.scalar.activation(warm[:], warm[:], AF.Derivative_Erf)

            # PSUM ring: RING slots of 2 banks; chunk at position ci uses
            # slot ci % RING, pairs of slots are evacuated with one op.
            ring = ps.tile([P, RING, 1024], f32, name="ring", bufs=1)

            # Warm the PE HAM clock gate: junk matmuls while the real
            # inputs are still loading, so real matmuls run at 2.4GHz.
            # They write the LAST ring slot so the first real chunks
            # (slots 0..RING-2) don't serialize behind them.
            junk = constp.tile([P, 512], fp16)
            nc.vector.memset(junk[:], 0.0)
            for _ in range(3):
                nc.tensor.matmul(ring[:, RING - 1, 0:512], junk[:, 0:P],
                                 junk[:], start=True, stop=True)

            # trow (frame indices 1..T): first 512 DMA'd from the host
            # (cheap, off the compute critical path), DVE doubling extends.
            trow = constp.tile([P, T], f32)
            nc.sync.dma_start(out=trow[:, 0:512], in_=trow0_d)
            for q0 in (512, 1024, 2048):
                nc.vector.tensor_scalar_add(
                    trow[:, q0:2 * q0], trow[:, 0:q0], float(q0)
                )

            # All input DMAs up front on the Sync queue, before any output
            # issue can block them (the queue drains in program order).
            tiles = []
            for b in range(BPC):
                bcol = colp.tile([P, KC], f32, name="bcol", tag="bcol")
                nc.sync.dma_start(out=bcol[:], in_=bias_d[b])
                xw = sb.tile([P, KC, DW], fp16, name="xw_t", tag="xw_t")
                xw_src = xw_d[b].rearrange("(k p) d -> p k d", p=P)
                for k in range(KC):
                    nc.sync.dma_start(
                        out=xw[:, k:k + 1, :], in_=xw_src[:, k:k + 1, :]
                    )
                tiles.append((bcol, xw))

            score_tiles = [
                sb.tile([P, KC, T], fp16, name="scores", tag="scores")
                for _ in range(BPC)
            ]

            def issue_piece(b, pi):
                bcol, _ = tiles[b]
                k, t0, t1 = pieces2[b][pi]
                nc.scalar.activation(
                    score_tiles[b][:, k, t0:t1], trow[:, t0:t1],
                    AF.Derivative_Erf, bias=bcol[:, k:k + 1], scale=rsv,
                )

            ot = None
            for ci, (b, m) in enumerate(chunk_seq):
                for pb, pi in issue_at.get(ci, ()):
                    issue_piece(pb, pi)

                bcol, xw = tiles[b]
                scores = score_tiles[b]
                sub = ci % GRP
                slot = ci % RING

                ks = mk2[b][m]
                for i, k in enumerate(ks):
                    lhsT = scores[:, k, m * P:(m + 1) * P]
                    st = (i == 0)
                    sp = (i == len(ks) - 1)
                    mma = nc.tensor.matmul(
                        ring[:, slot, 0:N0], lhsT, xw[:, k, 0:N0],
                        start=st, stop=sp,
                    )
                    mmb = nc.tensor.matmul(
                        ring[:, slot, N0:DW], lhsT, xw[:, k, N0:DW],
                        start=st, stop=sp,
                    )
                    add_dep_helper(mmb.ins, mma.ins,
                                   reason="keep N-pieces adjacent")

                if sub == 0:
                    ot = outp.tile([P, GRP, DW], fp16, name="ot", tag="ot")
                if eng_seq[ci] == "V":
                    nc.vector.tensor_copy(
                        ot[:, sub, :], ring[:, slot, 0:DW])
                else:
                    nc.scalar.activation(
                        ot[:, sub, :], ring[:, slot, 0:DW], AF.Copy)

                last_grp = ci >= len(chunk_seq) - 2 * GRP
                if last_grp and sub % 2 == 1:
                    # split the final group's DMA so its wire starts
                    # right after the first half's evacs
                    m0 = m - 1
                    nc.sync.dma_start(
                        out=out_d[b, m0 * P:(m0 + 2) * P, :]
                        .rearrange("(g p) d -> p g d", p=P),
                        in_=ot[:, sub - 1:sub + 1, :],
                    )
                elif not last_grp and sub == GRP - 1:
                    m0 = m - (GRP - 1)
                    nc.sync.dma_start(
                        out=out_d[b, m0 * P:(m0 + GRP) * P, :]
                        .rearrange("(g p) d -> p g d", p=P),
                        in_=ot[:],
                    )

    nc.compile()
    return nc


def _get_program(bands):
    prog = _PROGRAMS.get(bands)
    if prog is None:
        prog = _build_program(bands)
        _PROGRAMS[bands] = prog
    return prog


def _prepare(x, d, mask):
    x = np.asarray(x, dtype=np.float32)
    d64 = np.asarray(d, dtype=np.float64)
    mask = np.asarray(mask, dtype=bool)

    e = np.cumsum(d64, axis=-1)
    c = e - 0.5 * d64                      # (B, S) token centers
    c_m = np.where(mask, c, np.nan)

    # Sort batches by valid length; slot 0 takes the 8 shortest, slot 1 the
    # 8 longest. Similar lengths per slot give much tighter per-slot bands.
    order = np.argsort(mask.sum(1), kind="stable")
    bands2 = tuple(
        _compute_bands(c_m[order[s * NCORES:(s + 1) * NCORES]])
        for s in range(BPC)
    )

    c = np.where(mask, c, -1.0e4)          # masked tokens: exp underflows to 0
    bias = (-c / math.sqrt(VARIANCE)).astype(np.float32)
    # p-major layout [B, P, KC] so the on-device DMA reads contiguously
    bias = np.ascontiguousarray(bias.reshape(B, KC, P).transpose(0, 2, 1))

    xw = np.empty((B, S, DW), dtype=np.float16)
    xw[:, :, :D] = (x * XSCALE).astype(np.float16)
    xw[:, :, D] = XSCALE

    trow0 = np.broadcast_to(
        np.arange(1, 513, dtype=np.float32)[None, :], (P, 512)
    )
    trow0 = np.ascontiguousarray(trow0)

    in_maps = []
    for core in range(NCORES):
        idx = [order[core], order[NCORES + core]]
        in_maps.append({
            "xw": np.ascontiguousarray(xw[idx]),
            "bias": np.ascontiguousarray(bias[idx]),
            "trow0": trow0,
        })
    return in_maps, bands2, order


def run(x, d, mask, frame_length, trace=False):
    assert int(frame_length) == T
    in_maps, bands2, order = _prepare(x, d, mask)
    nc = _get_program(bands2)
    res = None
    for attempt in range(3):
        try:
            res = run_bass_kernel_spmd(nc, in_maps, list(range(NCORES)),
                                       trace=trace)
            break
        except Exception:
            # The first execution after a fresh compile occasionally hits a
            # transient device error; retrying succeeds.
            if attempt == 2:
                raise
    out = np.empty((B, T, D), dtype=np.float32)
    for core in range(NCORES):
        for s in range(BPC):
            nd = res.results[core]["out"][s].astype(np.float32)
            out[order[s * NCORES + core]] = nd[:, 0:D] / nd[:, D:DW]
    return out, res


def kernel(x, d, mask, frame_length):
    out, _ = run(x, d, mask, frame_length, trace=False)
    return out

